# revision 10
# baseline (speedup 1.0000x reference)
"""Trainium2 Bass kernel for nn_AggrHGraphConvWindow_79285096284407.

Pipeline: hetero GraphConv (3 small graphs, per-timestep weights) ->
leaky_relu -> concat -> 2-layer LSTM (H=256) over T=32 timesteps,
batch = 2000 rows.

Strategy:
  * CPU (cheap, sparse): build the normalized adjacency, compute
    agg = A_hat @ feat per conv (three small BLAS gemms), and fold the
    per-row-type conv weight selection + bias into a single dense
    [K=195(pad 256), 128] matmul per timestep by appending type-mask
    rows (bias) to the aggregated features.
  * Device (8 NeuronCores, SPMD, data-parallel over the 2000 rows,
    250 rows/core padded to 256): per-t conv matmul + leaky relu, then
    both LSTM layers fully on-chip in feature-major layout
    ([feature, batch] tiles) so the recurrence needs no transposes.
    float32r matmuls (1 cycle/row at moving dim >= 256).

Everything is hardcoded for the spec shapes; kernel() takes full inputs
and returns the full [2000, 32, 256] float32 output.
"""

import os
from contextlib import ExitStack

import numpy as np

N_NODE, N_POD, N_SVC = 100, 1500, 400
T, F, IN, H = 32, 64, 128, 256
NTOT = N_NODE + N_POD + N_SVC  # 2000
NCORES = 8
NPC = NTOT // NCORES  # 250 rows per core
NP = 256  # padded rows per core (moving dim >= 256 for f32r full rate)
KC = 2  # conv contraction k-tiles (195 -> 256)
G4 = 4 * H  # 1024 gates

_BUILT = None  # (nc,) cache
LAST_RESULT = None  # BassKernelResults of the most recent run


def _build_program():
    import concourse.bass as bass
    import concourse.mybir as mybir
    import concourse.tile as tile
    from concourse import bacc

    DT = mybir.dt
    f32 = DT.float32
    f32r = DT.float32r
    AF = mybir.ActivationFunctionType
    ALU = mybir.AluOpType

    nc = bacc.Bacc(
        "TRN2", target_bir_lowering=False, debug=False, num_devices=NCORES
    )

    aggt_d = nc.declare_dram_parameter("aggt", [T, KC, 128, NP], f32r, False)
    wbt_d = nc.declare_dram_parameter("wbt", [T, KC, 128, IN], f32r, False)
    wih0_d = nc.declare_dram_parameter("wih0t", [IN, G4], f32r, False)
    whh0_d = nc.declare_dram_parameter("whh0t", [2, 128, G4], f32r, False)
    wih1_d = nc.declare_dram_parameter("wih1t", [2, 128, G4], f32r, False)
    whh1_d = nc.declare_dram_parameter("whh1t", [2, 128, G4], f32r, False)
    b0_d = nc.declare_dram_parameter("b0", [128, 8], f32, False)
    b1_d = nc.declare_dram_parameter("b1", [128, 8], f32, False)
    out_d = nc.declare_dram_parameter("out", [T, 2, 128, NP], f32r, True)

    with tile.TileContext(nc) as tc, ExitStack() as ctx:
        wpool = ctx.enter_context(tc.tile_pool(name="w", bufs=1))
        spool = ctx.enter_context(tc.tile_pool(name="state", bufs=1))
        xpool = ctx.enter_context(tc.tile_pool(name="x", bufs=1))
        inpool = ctx.enter_context(tc.tile_pool(name="in", bufs=3))
        apool = ctx.enter_context(tc.tile_pool(name="act", bufs=12))
        tpool = ctx.enter_context(tc.tile_pool(name="tmp", bufs=4))
        xps = ctx.enter_context(tc.tile_pool(name="xps", bufs=2, space="PSUM"))
        gps = ctx.enter_context(tc.tile_pool(name="gps", bufs=6, space="PSUM"))

        # Resident weights
        wih0 = wpool.tile([128, G4], f32r)
        nc.sync.dma_start(wih0[:], wih0_d[:])
        whh0 = wpool.tile([128, 2, G4], f32r)
        whh1 = wpool.tile([128, 2, G4], f32r)
        wih1 = wpool.tile([128, 2, G4], f32r)
        for k in range(2):
            nc.sync.dma_start(whh0[:, k, :], whh0_d[k])
            nc.sync.dma_start(wih1[:, k, :], wih1_d[k])
            nc.sync.dma_start(whh1[:, k, :], whh1_d[k])
        b0 = wpool.tile([128, 8], f32)
        nc.sync.dma_start(b0[:], b0_d[:])
        b1 = wpool.tile([128, 8], f32)
        nc.sync.dma_start(b1[:], b1_d[:])

        # States (persistent, updated in place each step; first write at t=0)
        h1 = spool.tile([128, 2, NP], f32r)
        c1 = spool.tile([128, 2, NP], f32)
        h2 = spool.tile([128, 2, NP], f32r)
        c2 = spool.tile([128, 2, NP], f32)

        # Conv: x_t^T = leaky_relu(WB_t^T @ aggB_t^T), stored [feat, t, n]
        x_sb = xpool.tile([128, T, NP], f32r)
        for t in range(T):
            at = inpool.tile([128, KC, NP], f32r, tag="aggt")
            nc.sync.dma_start(at[:], aggt_d[t])
            wt = inpool.tile([128, KC, IN], f32r, tag="wbt")
            nc.sync.dma_start(wt[:], wbt_d[t])
            xp = xps.tile([128, NP], f32)
            nc.tensor.matmul(
                xp[:], wt[:, 0, :], at[:, 0, :],
                start=True, stop=False,
            )
            nc.tensor.matmul(
                xp[:], wt[:, 1, :], at[:, 1, :],
                start=False, stop=True,
            )
            xr = tpool.tile([128, NP], f32, tag="xraw")
            nc.vector.tensor_copy(xr[:], xp[:])
            nc.vector.scalar_tensor_tensor(
                x_sb[:, t, :], xr[:], 0.01, xr[:], op0=ALU.mult, op1=ALU.max
            )

        def gsl(g):
            return bass.ts(g, 128)

        def lstm_step(x_tiles, whh, bb, h, c, acts_tag, first):
            # x_tiles: list of (lhsT_slice, rhs_slice) for the input part.
            # first=True: h/c are implicitly zero (skip recurrent matmuls,
            # c = i*g) — also how the states get initialized without memset.
            acts = []
            n_in = len(x_tiles)
            for g in range(8):
                ps = gps.tile([128, NP], f32, tag="g")
                for i, (wsl, xsl) in enumerate(x_tiles):
                    nc.tensor.matmul(
                        ps[:], wsl[:, gsl(g)], xsl,
                        start=(i == 0), stop=(first and i == n_in - 1),
                    )
                if not first:
                    for k in range(2):
                        nc.tensor.matmul(
                            ps[:], whh[:, k, gsl(g)], h[:, k, :],
                            start=False, stop=(k == 1),
                        )
                a = apool.tile([128, NP], f32, tag=acts_tag)
                func = AF.Tanh if g in (4, 5) else AF.Sigmoid
                nc.scalar.activation(a[:], ps[:], func, bias=bb[:, g:g + 1])
                acts.append(a)
            for k in range(2):
                if first:
                    nc.vector.tensor_mul(c[:, k, :], acts[0 + k][:], acts[4 + k][:])
                else:
                    ig = tpool.tile([128, NP], f32, tag="ig")
                    nc.vector.tensor_mul(ig[:], acts[0 + k][:], acts[4 + k][:])
                    nc.vector.tensor_mul(c[:, k, :], acts[2 + k][:], c[:, k, :])
                    nc.vector.tensor_add(c[:, k, :], c[:, k, :], ig[:])
                th = tpool.tile([128, NP], f32, tag="th")
                nc.scalar.activation(th[:], c[:, k, :], AF.Tanh)
                nc.vector.tensor_mul(h[:, k, :], acts[6 + k][:], th[:])

        for t in range(T):
            lstm_step([(wih0, x_sb[:, t, :])], whh0, b0, h1, c1, "a1",
                      first=(t == 0))
            lstm_step(
                [(wih1[:, 0, :], h1[:, 0, :]), (wih1[:, 1, :], h1[:, 1, :])],
                whh1, b1, h2, c2, "a2", first=(t == 0),
            )
            for k in range(2):
                nc.sync.dma_start(out_d[t, k], h2[:, k, :])

    nc.compile()
    return nc


def _prep_inputs(node_feat, pod_feat, svc_feat, W_svc, b_svc, W_in, b_in,
                 W_ni, b_ni, W_ih0, W_hh0, b_ih0, b_hh0, W_ih1, W_hh1,
                 b_ih1, b_hh1, svc_src, svc_dst, in_src, in_dst, ni_src,
                 ni_dst):
    f32 = np.float32

    def conv_agg(feat, src, dst, n_src, n_dst):
        src = np.asarray(src, np.int64)
        dst = np.asarray(dst, np.int64)
        deg_o = np.maximum(np.bincount(src, minlength=n_src), 1.0).astype(f32)
        deg_i = np.maximum(np.bincount(dst, minlength=n_dst), 1.0).astype(f32)
        A = np.zeros((n_dst, n_src), f32)
        np.add.at(A, (dst, src), deg_i[dst] ** -0.5 * deg_o[src] ** -0.5)
        return A @ np.asarray(feat, f32).reshape(n_src, T * F)

    agg_node = conv_agg(pod_feat, in_src, in_dst, N_POD, N_NODE)
    agg_pod = conv_agg(node_feat, ni_src, ni_dst, N_NODE, N_POD)
    agg_svc = conv_agg(svc_feat, svc_src, svc_dst, N_SVC, N_SVC)

    # aggB^T: [T, K=256, NTOT]; K rows: [agg(64)|1] per type block
    aggBT = np.zeros((T, KC * 128, NTOT), f32)
    aggBT[:, 0:64, 0:N_NODE] = agg_node.reshape(N_NODE, T, F).transpose(1, 2, 0)
    aggBT[:, 64, 0:N_NODE] = 1.0
    aggBT[:, 65:129, N_NODE:N_NODE + N_POD] = (
        agg_pod.reshape(N_POD, T, F).transpose(1, 2, 0))
    aggBT[:, 129, N_NODE:N_NODE + N_POD] = 1.0
    aggBT[:, 130:194, N_NODE + N_POD:] = (
        agg_svc.reshape(N_SVC, T, F).transpose(1, 2, 0))
    aggBT[:, 194, N_NODE + N_POD:] = 1.0

    WB = np.zeros((T, KC * 128, IN), f32)
    WB[:, 0:64] = np.asarray(W_in, f32)
    WB[:, 64] = np.asarray(b_in, f32)
    WB[:, 65:129] = np.asarray(W_ni, f32)
    WB[:, 129] = np.asarray(b_ni, f32)
    WB[:, 130:194] = np.asarray(W_svc, f32)
    WB[:, 194] = np.asarray(b_svc, f32)
    wbt = np.ascontiguousarray(WB.reshape(T, KC, 128, IN))

    wih0t = np.ascontiguousarray(np.asarray(W_ih0, f32).T)  # [128, 1024]
    whh0t = np.ascontiguousarray(np.asarray(W_hh0, f32).T).reshape(2, 128, G4)
    wih1t = np.ascontiguousarray(np.asarray(W_ih1, f32).T).reshape(2, 128, G4)
    whh1t = np.ascontiguousarray(np.asarray(W_hh1, f32).T).reshape(2, 128, G4)
    b0 = np.ascontiguousarray(
        (np.asarray(b_ih0, f32) + np.asarray(b_hh0, f32)).reshape(8, 128).T)
    b1 = np.ascontiguousarray(
        (np.asarray(b_ih1, f32) + np.asarray(b_hh1, f32)).reshape(8, 128).T)

    in_maps = []
    for c in range(NCORES):
        a = np.zeros((T, KC * 128, NP), f32)
        a[:, :, :NPC] = aggBT[:, :, c * NPC:(c + 1) * NPC]
        in_maps.append({
            "aggt": a.reshape(T, KC, 128, NP),
            "wbt": wbt,
            "wih0t": wih0t,
            "whh0t": whh0t,
            "wih1t": wih1t,
            "whh1t": whh1t,
            "b0": b0,
            "b1": b1,
        })
    return in_maps


def _run(in_maps, trace=False):
    global _BUILT, LAST_RESULT
    from concourse.bass_utils import run_bass_kernel_spmd

    if _BUILT is None:
        _BUILT = _build_program()
    nc = _BUILT
    res = run_bass_kernel_spmd(nc, in_maps, list(range(NCORES)), trace=trace)
    LAST_RESULT = res
    return res


def kernel(**inputs) -> np.ndarray:
    in_maps = _prep_inputs(**inputs)
    trace = bool(os.environ.get("KERNEL_TRACE"))
    res = _run(in_maps, trace=trace)
    out = np.empty((NTOT, T, H), np.float32)
    for c in range(NCORES):
        r = np.asarray(res.results[c]["out"]).reshape(T, H, NP)
        out[c * NPC:(c + 1) * NPC] = r[:, :, :NPC].transpose(2, 0, 1)
    return out
